# revision 10
# baseline (speedup 1.0000x reference)
"""CronRootAttention (causal sqrt-N sparse attention + GQA projections) on 8 TRN2 cores.

Sharding: pure sequence shard — each core owns 256 queries, computes all 16
heads for them. Weights replicated; kv projections computed per-core for the
local 320-key span plus the 44 shared strided keys.

v3 dataflow (vs v2):
  - Local attention in 64-query blocks: each block's 46-wide causal band fits
    one [128-key, 64-query] tile, so scores go from 1536 to 1024 cols/g and a
    single core-independent additive mask [128, 4h*64] serves every block
    (PE identity preload per tile).
  - Strided mask is multiplicative post-exp on DVE (bf16 2x mode), not a PE
    preload.
  - PV stationaries are v row-tiles at two 64-shifted alignments (two extra
    PSUM->SBUF half copies instead of extra matmul cols).
  - Output projection is interleaved into the attention g-loop: 4 y-chunk
    PSUM accumulators live across phase B and receive each g's two k-tiles
    as soon as that g's heads are normalized; only the last head-pair's
    2048 cols remain after the final normalize.
  - Inputs packed into 5 dram tensors in k-tile-interleaved layout, DMA'd in
    dependency order (10 DMAs; first matmul needs only the first one).
"""

import math
import os
import sys

sys.path.insert(0, "/opt/trn_rl_repo")

import numpy as np
import ml_dtypes
import concourse.bass as bass
import concourse.tile as tile
from concourse import bacc, mybir
from concourse.bass_utils import run_bass_kernel_spmd

F32 = mybir.dt.float32
BF16 = mybir.dt.bfloat16
EXP = mybir.ActivationFunctionType.Exp
COPY = mybir.ActivationFunctionType.Copy
MULT = mybir.AluOpType.mult

MM_DT = BF16
NP_DT = ml_dtypes.bfloat16

# Problem constants (hardcoded per contract).
B, S, D = 1, 2048, 1024
H, H_KV, HD = 16, 4, 64
W = int(math.ceil(math.sqrt(S)))  # 46
NCORES = 8
SQ = S // NCORES  # 256 queries per core
KSPAN = 320  # local key span: [qs-64, qs+256)
SIDX = np.arange(W - 1, S, W)  # strided key positions
NS = len(SIDX)  # 44
KT = D // 128  # 8 contraction k-tiles
MASKV = -400.0  # additive mask value; exp(0.125*(qk-400)) == 0

# packed t_a column layout per k-tile: [xkv 320 | wk 256 | wv 260]
TA_W = KSPAN + 256 + 260  # 836
# packed aux layout: [xs 8*44 | ident 128 | mloc0 256 | mloc1 256]
AUX_XS, AUX_ID, AUX_M0, AUX_M1 = 0, 352, 480, 736
AUX_W = 992


def build_nc():
    nc = bacc.Bacc("TRN2", target_bir_lowering=False, debug=False, num_devices=1)
    ta = nc.dram_tensor("ta", [128, KT, TA_W], MM_DT, kind="ExternalInput").ap()
    wq = nc.dram_tensor("wq", [128, KT, D], MM_DT, kind="ExternalInput").ap()
    wo = nc.dram_tensor("wo", [128, KT, D], MM_DT, kind="ExternalInput").ap()
    aux = nc.dram_tensor("aux", [128, AUX_W], MM_DT, kind="ExternalInput").ap()
    ms01 = nc.dram_tensor("ms01", [NS, 1024], MM_DT, kind="ExternalInput").ap()
    y = nc.dram_tensor("y", [SQ, D], MM_DT, kind="ExternalOutput").ap()

    with tile.TileContext(nc) as tc:
        with (
            tc.tile_pool(name="consts", bufs=1) as consts,
            tc.tile_pool(name="work", bufs=1) as work,
        ):
            ta_sb = consts.tile([128, KT, TA_W], MM_DT)
            wq_sb = consts.tile([128, KT, D], MM_DT)
            wo_sb = consts.tile([128, KT, D], MM_DT)
            aux_sb = consts.tile([128, AUX_W], MM_DT)
            ms01_sb = consts.tile([NS, 1024], MM_DT)

            # DMAs in dependency order; first matmul needs only ta kt0.
            nc.sync.dma_start(out=ta_sb[:, 0:1, :], in_=ta[:, 0:1])
            nc.sync.dma_start(out=ta_sb[:, 1:2, :], in_=ta[:, 1:2])
            nc.sync.dma_start(out=ta_sb[:, 2:4, :], in_=ta[:, 2:4])
            nc.sync.dma_start(out=ta_sb[:, 4:8, :], in_=ta[:, 4:8])
            nc.sync.dma_start(out=aux_sb[:], in_=aux)
            nc.sync.dma_start(out=wq_sb[:, 0:4, :], in_=wq[:, 0:4])
            nc.sync.dma_start(out=wq_sb[:, 4:8, :], in_=wq[:, 4:8])
            nc.sync.dma_start(out=ms01_sb[:], in_=ms01)
            nc.sync.dma_start(out=wo_sb[:, 0:4, :], in_=wo[:, 0:4])
            nc.sync.dma_start(out=wo_sb[:, 4:8, :], in_=wo[:, 4:8])

            def xkv(kt):
                return ta_sb[:, kt, 0:KSPAN]

            def wk_sl(kt, ot):
                return ta_sb[:, kt, KSPAN + 128 * ot : KSPAN + 128 * ot + 128]

            def wv_sl(kt):
                return ta_sb[:, kt, KSPAN + 256 : KSPAN + 256 + 260]

            xs_v = aux_sb[:, AUX_XS : AUX_XS + KT * NS].rearrange(
                "p (kt s) -> p kt s", kt=KT
            )
            id_sb = aux_sb[:, AUX_ID : AUX_ID + 128]
            mloc = [
                aux_sb[:, AUX_M0 : AUX_M0 + 256],
                aux_sb[:, AUX_M1 : AUX_M1 + 256],
            ]

            # work tiles (attention matmul operands all at base partition 0)
            q_sb = work.tile([64, H, SQ], MM_DT)  # q_T per head
            k_sb = work.tile([64, 4, KSPAN], MM_DT)  # k_T per kv head
            ks_sb = work.tile([64, 4, NS], MM_DT)
            v_sb = work.tile([128, 4, 260], MM_DT)  # 4 stationary alignments
            vs_sb = work.tile([NS, 260], MM_DT)
            attn_sb = work.tile([128, KT, SQ], MM_DT)

            _eng = [0]

            def copy_any(out, in_):
                e = _eng[0] % 2
                _eng[0] += 1
                if e == 0:
                    nc.scalar.activation(out, in_, COPY)
                else:
                    nc.vector.tensor_copy(out, in_)

            # ---- phase A wave 1: k and v projections (per k-tile) ----
            with tc.tile_pool(name="psA1", bufs=1, space="PSUM") as psA1:
                kps = [psA1.tile([128, 512], F32, tag="kp", bufs=2, name=f"kp{_}") for _ in range(2)]
                vps = [psA1.tile([128, 512], F32, tag="vp", bufs=3, name=f"vp{_}") for _ in range(3)]
                for kt in range(KT):
                    st, sp = kt == 0, kt == KT - 1
                    for ot in range(2):
                        nc.tensor.matmul(
                            kps[ot][:, 0:KSPAN], wk_sl(kt, ot), xkv(kt),
                            start=st, stop=sp,
                        )
                    for mt in range(2):
                        nc.tensor.matmul(
                            vps[mt][:, 0:260],
                            ta_sb[:, kt, 128 * mt : 128 * mt + 128],
                            wv_sl(kt),
                            start=st, stop=sp,
                        )
                    nc.tensor.matmul(
                        vps[2][0:64, 0:260], ta_sb[:, kt, 256:320], wv_sl(kt),
                        start=st, stop=sp,
                    )
                for ot in range(2):
                    copy_any(k_sb[:, 2 * ot, :], kps[ot][0:64, 0:KSPAN])
                    copy_any(k_sb[:, 2 * ot + 1, :], kps[ot][64:128, 0:KSPAN])
                # v alignments: tiles cover span rows [64b, 64b+128)
                copy_any(v_sb[:, 0, :], vps[0][:, 0:260])
                copy_any(v_sb[:, 2, :], vps[1][:, 0:260])
                copy_any(v_sb[0:64, 1, :], vps[0][64:128, 0:260])
                copy_any(v_sb[64:128, 1, :], vps[1][0:64, 0:260])
                copy_any(v_sb[0:64, 3, :], vps[1][64:128, 0:260])
                copy_any(v_sb[64:128, 3, :], vps[2][0:64, 0:260])
                for t in range(4):
                    ones_cols = v_sb[:, t, :].rearrange("p (g c) -> p g c", g=4)[
                        :, :, 64
                    ]
                    nc.gpsimd.memset(ones_cols, 1.0)

            # ---- phase A wave 2: ks, vs, q projections (q in 2 passes of 4
            # accumulators; PSUM slots are bank-granular, 8 banks total) ----
            with tc.tile_pool(name="psA2", bufs=1, space="PSUM") as psA2:
                vsp = psA2.tile([128, 512], F32, tag="vsp")
                ksps = [psA2.tile([128, 64], F32, tag="ksp", bufs=2, name=f"ksp{_}") for _ in range(2)]
                for qpass in range(2):
                    qps = [
                        psA2.tile([128, SQ], F32, tag="qp", bufs=4, name=f"qp{qpass}{_}")
                        for _ in range(4)
                    ]
                    for kt in range(KT):
                        st, sp = kt == 0, kt == KT - 1
                        if qpass == 0:
                            for ot in range(2):
                                nc.tensor.matmul(
                                    ksps[ot][:, 0:NS], wk_sl(kt, ot), xs_v[:, kt, :],
                                    start=st, stop=sp,
                                )
                            nc.tensor.matmul(
                                vsp[0:NS, 0:260], xs_v[:, kt, :], wv_sl(kt),
                                start=st, stop=sp,
                            )
                        for i in range(4):
                            ot = 4 * qpass + i
                            nc.tensor.matmul(
                                qps[i][:],
                                wq_sb[:, kt, 128 * ot : 128 * ot + 128],
                                xkv(kt)[:, 64:320],
                                start=st, stop=sp,
                            )
                    if qpass == 0:
                        for ot in range(2):
                            copy_any(ks_sb[:, 2 * ot, :], ksps[ot][0:64, 0:NS])
                            copy_any(ks_sb[:, 2 * ot + 1, :], ksps[ot][64:128, 0:NS])
                        copy_any(vs_sb[:], vsp[0:NS, 0:260])
                        vs_ones = vs_sb[:].rearrange("p (g c) -> p g c", g=4)[
                            :, :, 64
                        ]
                        nc.gpsimd.memset(vs_ones, 1.0)
                    for i in range(4):
                        ot = 4 * qpass + i
                        copy_any(q_sb[:, 2 * ot, :], qps[i][0:64, :])
                        copy_any(q_sb[:, 2 * ot + 1, :], qps[i][64:128, :])

            # ---- phase B (attention) + interleaved phase C (output proj) ----
            def q_sl(h, c0, c1):
                return q_sb[:, h, c0:c1]

            def k_sl(g, b):
                return k_sb[:, g, 64 * b : 64 * b + 128]

            def ks_sl(g):
                return ks_sb[:, g, :]

            with (
                tc.tile_pool(name="ps_y", bufs=1, space="PSUM") as psy,
                tc.tile_pool(name="ps_blk", bufs=1, space="PSUM") as psb,
                tc.tile_pool(name="ptiles", bufs=1) as pt,
                tc.tile_pool(name="small", bufs=1) as sm,
                tc.tile_pool(name="yout", bufs=2) as yo,
            ):
                # y chunk accumulators: (query-tile st, output half c0)
                CHUNKS = [(0, 0), (0, 512), (1, 0), (1, 512)]
                yts = [psy.tile([128, 512], F32, tag=f"y{i}", name=f"yt{i}") for i in range(4)]

                def emit_C(g):
                    # contraction k-tiles for this g's heads; last g reversed so
                    # the final exposed work depends on the earliest-normalized
                    # head pair.
                    kts = (2 * g, 2 * g + 1) if g < 3 else (7, 6)
                    for i, (qt, c0) in enumerate(CHUNKS):
                        for kt in kts:
                            nc.tensor.matmul(
                                yts[i][:],
                                attn_sb[:, kt, 128 * qt : 128 * qt + 128],
                                wo_sb[:, kt, c0 : c0 + 512],
                                start=kt == 0,
                                stop=kt == 6,
                                skip_group_check=True,
                            )

                for g in range(4):
                    # strided scores: 2 head-pair tiles (no preload;
                    # multiplicative mask post-exp). All phase-B score/PV
                    # tiles share one 4-bank ring (tag "blk").
                    pstrs = []
                    for u in range(2):
                        st = psb.tile([NS, 512], F32, tag="blk", bufs=4,
                                      name=f"st{u}")
                        for i in range(2):
                            nc.tensor.matmul(
                                st[:, 256 * i : 256 * i + 256],
                                ks_sl(g),
                                q_sl(4 * g + 2 * u + i, 0, SQ),
                                start=True, stop=True,
                                skip_group_check=True,
                            )
                        pstr = pt.tile([NS, 512], MM_DT, tag=f"pstr{u}", bufs=2)
                        nc.scalar.activation(pstr[:], st[:], EXP, scale=0.125)
                        nc.vector.tensor_tensor(
                            out=pstr[:],
                            in0=pstr[:],
                            in1=ms01_sb[:, 512 * u : 512 * u + 512],
                            op=MULT,
                        )
                        pstrs.append(pstr)
                    # local blocks: 64 queries vs 128-key window
                    pbs = []
                    for b in range(4):
                        sp = psb.tile([128, 256], F32, tag="blk", bufs=4,
                                      name=f"sp{b}")
                        nc.tensor.matmul(
                            sp[:], id_sb, mloc[0 if b == 0 else 1],
                            start=True, stop=False, skip_group_check=True,
                        )
                        for hh in range(4):
                            nc.tensor.matmul(
                                sp[:, 64 * hh : 64 * hh + 64],
                                k_sl(g, b),
                                q_sl(4 * g + hh, 64 * b, 64 * b + 64),
                                start=False, stop=hh == 3,
                                skip_group_check=True,
                            )
                        pb = pt.tile([128, 256], MM_DT, tag=f"p{b}", bufs=2)
                        nc.scalar.activation(pb[:], sp[:], EXP, scale=0.125)
                        pbs.append(pb)
                    # fill the exp->PV latency with the previous g's y k-tiles
                    if g > 0:
                        emit_C(g - 1)
                    # PV + normalize per head; last g normalizes heads 2,3
                    # first so emit_C(3)'s kt7 unblocks earliest.
                    order = (0, 1, 2, 3) if g < 3 else (2, 3, 0, 1)
                    for hh in order:
                        h = 4 * g + hh
                        pv = psb.tile([65, SQ], F32, tag="blk", bufs=4,
                                      name=f"pv{hh}")
                        nc.tensor.matmul(
                            pv[:],
                            vs_sb[:, 65 * g : 65 * g + 65],
                            pstrs[hh // 2][:, 256 * (hh % 2) : 256 * (hh % 2) + 256],
                            start=True, stop=False,
                        )
                        for b in range(4):
                            nc.tensor.matmul(
                                pv[:, 64 * b : 64 * b + 64],
                                v_sb[:, b, 65 * g : 65 * g + 65],
                                pbs[b][:, 64 * hh : 64 * hh + 64],
                                start=False, stop=b == 3,
                            )
                        rt = sm.tile([1, SQ], F32, tag="rt", bufs=2)
                        nc.vector.reciprocal(rt[:], pv[64:65, :])
                        rep = sm.tile([64, SQ], F32, tag="rep", bufs=2)
                        nc.gpsimd.partition_broadcast(rep[:], rt[:], channels=64)
                        nc.vector.tensor_tensor(
                            out=attn_sb[64 * (h % 2) : 64 * (h % 2) + 64, h // 2, :],
                            in0=pv[0:64, :],
                            in1=rep[:],
                            op=MULT,
                        )
                emit_C(3)
                for i, (qt, c0) in enumerate(CHUNKS):
                    ys = yo.tile([128, 512], MM_DT, tag="ys")
                    if i % 2 == 0:
                        nc.scalar.activation(ys[:], yts[i][:], COPY)
                    else:
                        nc.vector.tensor_copy(ys[:], yts[i][:])
                    nc.sync.dma_start(
                        out=y[128 * qt : 128 * qt + 128, c0 : c0 + 512], in_=ys[:]
                    )
    nc.compile()
    return nc


def host_prep(x, Wq, Wk, Wv, Wo):
    """Build per-core input maps (pure data reordering, no FLOPs)."""
    x2 = np.asarray(x, np.float32).reshape(S, D)
    xT = np.ascontiguousarray(x2.T)  # [D, S]
    xpad = np.zeros((D, 64 + S), np.float32)
    xpad[:, 64:] = xT
    xs = xT[:, SIDX]  # [D, 44]
    wkT = np.asarray(Wk, np.float32).T  # [D, 256]
    wvT = np.asarray(Wv, np.float32).T  # [D, 256]
    wv = np.zeros((D, 260), np.float32)
    for g in range(4):
        wv[:, 65 * g : 65 * g + 64] = wvT[:, 64 * g : 64 * g + 64]
    wq_t = np.ascontiguousarray(
        np.asarray(Wq, np.float32).T.reshape(KT, 128, D).transpose(1, 0, 2)
    ).astype(NP_DT)
    wo_t = np.ascontiguousarray(
        np.asarray(Wo, np.float32).T.reshape(KT, 128, D).transpose(1, 0, 2)
    ).astype(NP_DT)

    # local band mask (core-independent except core 0's first block):
    # i = qb + c, j = qb - 64 + r -> valid iff 0 <= c + 64 - r <= 45
    r = np.arange(128)[:, None]
    c = np.arange(64)[None, :]
    band = (c + 64 - r >= 0) & (c + 64 - r <= 45)
    mloc1 = np.tile(np.where(band, 0.0, MASKV).astype(np.float32), (1, 4))

    ident = np.eye(128, dtype=np.float32)

    in_maps = []
    for core in range(NCORES):
        qs = SQ * core
        xkv = xpad[:, qs : qs + KSPAN]  # [D, 320]
        # t_a: [128, KT, 836] = per-kt [xkv | wk | wv]
        ta = np.concatenate(
            [
                xkv.reshape(KT, 128, KSPAN),
                wkT.reshape(KT, 128, 256),
                wv.reshape(KT, 128, 260),
            ],
            axis=2,
        ).transpose(1, 0, 2)
        ta = np.ascontiguousarray(ta).astype(NP_DT)
        # block-0 mask additionally kills keys with global j < 0 (core 0)
        jglob = qs - 64 + np.arange(128)[:, None]
        band0 = band & (jglob >= 0)
        mloc0 = np.tile(np.where(band0, 0.0, MASKV).astype(np.float32), (1, 4))
        auxa = np.zeros((128, AUX_W), np.float32)
        auxa[:, AUX_XS : AUX_XS + KT * NS] = (
            xs.reshape(KT, 128, NS).transpose(1, 0, 2).reshape(128, KT * NS)
        )
        auxa[:, AUX_ID : AUX_ID + 128] = ident
        auxa[:, AUX_M0 : AUX_M0 + 256] = mloc0
        auxa[:, AUX_M1 : AUX_M1 + 256] = mloc1
        # strided multiplicative mask: valid iff sidx <= (qs + c) - 46
        ii = qs + np.arange(SQ)[None, :]
        ms = (SIDX[:, None] <= ii - W).astype(np.float32)
        ms01 = np.ascontiguousarray(np.tile(ms, (1, 4))).astype(NP_DT)
        in_maps.append(
            {
                "ta": ta,
                "wq": wq_t,
                "wo": wo_t,
                "aux": auxa.astype(NP_DT),
                "ms01": ms01,
            }
        )
    return in_maps


_NC_CACHE = {}


def get_nc():
    if "nc" not in _NC_CACHE:
        _NC_CACHE["nc"] = build_nc()
    return _NC_CACHE["nc"]


def kernel(x, Wq, Wk, Wv, Wo):
    nc = get_nc()
    in_maps = host_prep(x, Wq, Wk, Wv, Wo)
    res = run_bass_kernel_spmd(nc, in_maps, core_ids=list(range(NCORES)))
    yrows = np.concatenate([r["y"] for r in res.results], axis=0)  # [S, D]
    return np.ascontiguousarray(yrows).reshape(B, S, D).astype(np.float32)
